# revision 11
# baseline (speedup 1.0000x reference)
"""Trainium2 Bass kernel for nn_ExpressionEstimator_Attention.

Data-parallel across 8 NeuronCores: each core processes B/8 = 4096 samples.
Per-core pipeline (activations kept as (C_partitions, b, t) SBUF tiles):
  x load -> PE transpose to channel-major -> conv1..4 (per-t matmuls, 3 taps
  accumulated in PSUM) -> DRAM-roundtrip reshape (the torch-faithful
  (B,64,9)->(B*9,64) row-major remix) -> fc1..3 -> att conv stack (taps
  K-stacked via shifted SBUF replicas, one matmul per t) -> logits matmul ->
  exp -> transpose to batch-partition layout -> softmax-normalized attention
  contraction -> indirect-DMA gather of mapping rows -> final einsum on DVE.

Leaky ReLU (exact, no Lrelu table): r = Identity(0.02*psum + 0.02*b) on ACT,
out = (psum + b) max r on DVE — one op per engine, bias via per-partition APs.
"""

import os
import sys

for _p in ("/opt/trn_rl_repo", "/root/.axon_site/_ro/trn_rl_repo"):
    if os.path.isdir(_p) and _p not in sys.path:
        sys.path.insert(0, _p)

import numpy as np

import concourse.bass as bass
import concourse.mybir as mybir
import concourse.tile as tile
from concourse import bacc
from concourse.masks import make_identity

FP = mybir.dt.float32
AX = mybir.AxisListType.X
OP = mybir.AluOpType
AF = mybir.ActivationFunctionType

NCORES = 8
B_FULL = 32768
T = 9
NA = 64
S = 32
NE = 53
NID = 5000

# (name, Cin, Cout) for the main conv stack
CONVS = [("c1", 64, 32), ("c2", 32, 64), ("c3", 64, 128), ("c4", 128, 64)]
FCS = [("f1", 64, 128, "leaky"), ("f2", 128, 64, "leaky"), ("f3", 64, 32, "tanh")]
ATTS = [("a1", 32, 16), ("a2", 16, 8), ("a3", 8, 4), ("a4", 4, 2), ("a5", 2, 1)]


def leaky_act(nc, rt, pm, out, bias2):
    """out = leaky(pm + b)  with bias2 = [[b, 0.02b]] per-partition.

    rt (SBUF scratch) <- ACT Identity(0.02*pm + 0.02*b)
    out <- DVE (pm + b) max rt
    """
    nc.scalar.activation(rt, pm, AF.Identity, scale=0.02, bias=bias2[:, 1:2])
    nc.vector.scalar_tensor_tensor(
        out=out, in0=pm, scalar=bias2[:, 0:1], in1=rt, op0=OP.add, op1=OP.max
    )


def emit(tc, io, B_core, NB):
    """Emit the per-core program. io: dict name -> DRAM AP."""
    nc = tc.nc
    NCH = B_core // NB
    J = NB * T // 128
    assert NB * T % 128 == 0 and NB % 128 == 0
    NBU = NB // 128
    FLAT = NB * T

    import contextlib

    with contextlib.ExitStack() as ctx:
        const = ctx.enter_context(tc.tile_pool(name="const", bufs=1))
        xr_p = ctx.enter_context(tc.tile_pool(name="xr", bufs=2))
        st_p = ctx.enter_context(tc.tile_pool(name="stage", bufs=6))
        rt_p = ctx.enter_context(tc.tile_pool(name="rt", bufs=3))
        sm_p = ctx.enter_context(tc.tile_pool(name="small", bufs=2))
        m_p = ctx.enter_context(tc.tile_pool(name="gath", bufs=2))
        ob_p = ctx.enter_context(tc.tile_pool(name="outb", bufs=2))
        dram = ctx.enter_context(tc.tile_pool(name="dram", bufs=2, space="DRAM"))
        rep_ps = {
            nm: ctx.enter_context(tc.tile_pool(name="repp_" + nm, bufs=1))
            for nm, _, _ in ATTS
        }
        ps_tr = ctx.enter_context(tc.tile_pool(name="ps_tr", bufs=2, space="PSUM"))
        ps_mm = ctx.enter_context(tc.tile_pool(name="ps_mm", bufs=3, space="PSUM"))
        ps_t2 = ctx.enter_context(tc.tile_pool(name="ps_t2", bufs=2, space="PSUM"))

        # ---------------- constants ----------------
        ident = const.tile([128, 128], FP)
        make_identity(nc, ident[:])

        wt = {}
        bt = {}
        for nm, ci, co in CONVS:
            wt[nm] = const.tile([ci, 3, co], FP, name="w_" + nm)
            nc.sync.dma_start(wt[nm][:], io["w" + nm][:])
            bt[nm] = const.tile([co, 2], FP, name="b_" + nm)
            nc.sync.dma_start(bt[nm][:], io["b" + nm][:])
        for nm, ci, co, _ in FCS:
            wt[nm] = const.tile([ci, co], FP, name="w_" + nm)
            nc.sync.dma_start(wt[nm][:], io["w" + nm][:])
            bt[nm] = const.tile([co, 2], FP, name="b_" + nm)
            nc.sync.dma_start(bt[nm][:], io["b" + nm][:])
        for nm, ci, co in ATTS:
            wt[nm] = const.tile([96, co], FP, name="w_" + nm)
            nc.sync.dma_start(wt[nm][:], io["w" + nm][:])
            bt[nm] = const.tile([co, 2], FP, name="b_" + nm)
            nc.sync.dma_start(bt[nm][:], io["b" + nm][:])
        lw_t = const.tile([T, T], FP)
        nc.sync.dma_start(lw_t[:], io["lwT"][:])
        lb_t = const.tile([T, 1], FP)
        nc.sync.dma_start(lb_t[:], io["lb2"][:])

        idx_t = const.tile([128, NCH * NBU], mybir.dt.int32)
        nc.sync.dma_start(
            idx_t[:],
            io["idx"].rearrange("(col p) -> p col", p=128),
        )

        x_rows = io["x"].rearrange("b t c -> (b t) c")
        out_d = io["out"]
        map_d = io["map"]

        # ---------------- chunk loop ----------------
        for c in range(NCH):
            # -- load x chunk, rows (b t) on partitions --
            xr = xr_p.tile([128, J, NA], FP)
            nc.sync.dma_start(
                xr[:],
                x_rows[c * FLAT : (c + 1) * FLAT, :].rearrange(
                    "(p j) c -> p j c", p=128
                ),
            )
            # -- transpose to X0 (64, t, b) --
            x0 = st_p.tile([NA, T, NB], FP, tag="stage")
            for g in range(J // 3):
                ptr = ps_tr.tile([NA, 3, 128], FP)
                for dj in range(3):
                    nc.tensor.transpose(
                        ptr[:, dj, :], xr[:, 3 * g + dj, :], ident[:128, :128]
                    )
                j0 = 3 * g
                bq, t0 = j0 // T, j0 % T
                dest = x0[:, t0 : t0 + 3, bq::NBU]
                nc.scalar.copy(out=dest, in_=ptr[:])

            # -- main convs --
            cur = x0
            for nm, ci, co in CONVS:
                nxt = st_p.tile([co, T, NB], FP, tag="stage")
                for t in range(T):
                    ks = [k for k in range(3) if 0 <= t + k - 1 <= T - 1]
                    pm = ps_mm.tile([co, NB], FP, tag="pm")
                    for i, k in enumerate(ks):
                        nc.tensor.matmul(
                            pm[:],
                            wt[nm][:, k, :],
                            cur[:, t + k - 1, :],
                            start=(i == 0),
                            stop=(i == len(ks) - 1),
                        )
                    rt = rt_p.tile([128, NB], FP, tag="rt")
                    leaky_act(nc, rt[:co, :], pm[:], nxt[:, t, :], bt[nm])
                cur = nxt

            # -- reshape via DRAM roundtrip: D2 row i = 9c+t = 64r+j --
            d2 = dram.tile([NA * T, NB], FP)
            nc.sync.dma_start(
                d2[:].rearrange("(c t) b -> c (t b)", t=T),
                cur[:].rearrange("c t b -> c (t b)"),
            )
            z = st_p.tile([64, T, NB], FP, tag="stage")
            nc.sync.dma_start(
                z[:],
                d2[:].rearrange("(r j) b -> j r b", j=64),
            )

            # -- fc stack on flat (r b) columns --
            curf = z[:].rearrange("j r b -> j (r b)")
            for nm, ci, co, act in FCS:
                nxt = st_p.tile([co, FLAT], FP, tag="stage")
                for q0 in range(0, FLAT, 512):
                    q1 = min(q0 + 512, FLAT)
                    pmt = ps_mm.tile([co, 512], FP, tag="pm")
                    pm = pmt[:, : q1 - q0]
                    nc.tensor.matmul(pm, wt[nm][:], curf[:, q0:q1], start=True, stop=True)
                    if act == "leaky":
                        rt = rt_p.tile([128, 512], FP, tag="rt")
                        leaky_act(nc, rt[:co, : q1 - q0], pm, nxt[:, q0:q1], bt[nm])
                    else:
                        nc.scalar.activation(
                            nxt[:, q0:q1], pm, AF.Tanh, bias=bt[nm][:, 0:1]
                        )
                curf = nxt[:]

            rs = curf.rearrange("s (r b) -> s r b", b=NB)  # (32, 9, NB)

            # -- attention convs: shifted K-stacked replicas, 1 matmul per t --
            prev = rs
            for nm, ci, co in ATTS:
                rep = rep_ps[nm].tile([96, T, NB], FP, tag="rep")
                if c == 0:
                    # zero whole tile once (boundary slices + inter-k pad rows);
                    # this pool slot is exclusive to this layer so zeros persist.
                    nc.gpsimd.memset(rep[:], 0.0)
                nc.sync.dma_start(rep[0:ci, 1:T, :], prev[:, 0 : T - 1, :])
                nc.sync.dma_start(rep[32 : 32 + ci, :, :], prev[:, :, :])
                nc.sync.dma_start(rep[64 : 64 + ci, 0 : T - 1, :], prev[:, 1:T, :])
                nxt = st_p.tile([co, T, NB], FP, tag="stage")
                for t in range(T):
                    pm = ps_mm.tile([co, NB], FP, tag="pm")
                    nc.tensor.matmul(pm[:], wt[nm][:], rep[:, t, :], start=True, stop=True)
                    rt = rt_p.tile([128, NB], FP, tag="rt")
                    leaky_act(nc, rt[:co, :], pm[:], nxt[:, t, :], bt[nm])
                prev = nxt[:]

            # -- logits: gather a into (9, NB) then one matmul --
            a5t = sm_p.tile([T, NB], FP, tag="a5t")
            nc.sync.dma_start(a5t[:], prev)
            pml = ps_mm.tile([T, NB], FP, tag="pm")
            nc.tensor.matmul(pml[:], lw_t[:], a5t[:], start=True, stop=True)
            et9 = sm_p.tile([T, NB], FP, tag="exp9")
            nc.scalar.activation(et9[:], pml[:], AF.Exp, bias=lb_t[:, 0:1])

            # -- per-128 subchunk: transpose to b-partitions, attention + output --
            for u in range(NBU):
                bsl = slice(u * 128, (u + 1) * 128)
                prt = ps_t2.tile([128, T, S], FP, tag="t2")
                for t in range(T):
                    nc.tensor.transpose(prt[:, t, :], rs[:, t, bsl], ident[:S, :S])
                pre = ps_t2.tile([128, T], FP, tag="t2")
                nc.tensor.transpose(pre[:], et9[:, bsl], ident[:T, :T])

                rsb = sm_p.tile([128, T, S], FP, tag="rsb")
                nc.vector.tensor_copy(rsb[:], prt[:])
                etb = sm_p.tile([128, T], FP, tag="etb")
                nc.scalar.copy(out=etb[:], in_=pre[:])

                den = sm_p.tile([128, 1], FP, tag="den")
                nc.vector.tensor_reduce(out=den[:], in_=etb[:], op=OP.add, axis=AX)
                rcp = sm_p.tile([128, 1], FP, tag="rcp")
                nc.vector.reciprocal(rcp[:], den[:])

                pp = sm_p.tile([128, S, T], FP, tag="pp")
                nc.vector.tensor_tensor(
                    out=pp[:].rearrange("p s t -> p t s"),
                    in0=rsb[:],
                    in1=etb[:].unsqueeze(2).broadcast_to([128, T, S]),
                    op=OP.mult,
                )
                sub_u = sm_p.tile([128, S], FP, tag="subu")
                nc.vector.tensor_reduce(out=sub_u[:], in_=pp[:], op=OP.add, axis=AX)
                sub_n = sm_p.tile([128, S], FP, tag="subn")
                nc.vector.tensor_scalar(
                    out=sub_n[:], in0=sub_u[:], scalar1=rcp[:, 0:1], scalar2=10.0,
                    op0=OP.mult, op1=OP.mult,
                )

                mt = m_p.tile([128, NE * S], FP, tag="mt")
                nc.gpsimd.indirect_dma_start(
                    out=mt[:],
                    out_offset=None,
                    in_=map_d[:],
                    in_offset=bass.IndirectOffsetOnAxis(
                        ap=idx_t[:, c * NBU + u : c * NBU + u + 1], axis=0
                    ),
                )
                pf = sm_p.tile([128, NE, S], FP, tag="pf")
                nc.vector.tensor_tensor(
                    out=pf[:],
                    in0=mt[:].rearrange("p (e s) -> p e s", s=S),
                    in1=sub_n[:].unsqueeze(1).broadcast_to([128, NE, S]),
                    op=OP.mult,
                )
                ob = ob_p.tile([128, NE], FP, tag="ob")
                nc.vector.tensor_reduce(out=ob[:], in_=pf[:], op=OP.add, axis=AX)
                nc.sync.dma_start(out_d[c * NB + u * 128 : c * NB + (u + 1) * 128, :], ob[:])


def prep_host_inputs(inputs, B_core):
    """Transform weights to device layouts. Returns (common dict, per-core fn)."""
    com = {}
    for i, (nm, ci, co) in enumerate(CONVS, start=1):
        w = np.asarray(inputs[f"cw{i}"])
        b = np.asarray(inputs[f"cb{i}"])
        com["w" + nm] = np.ascontiguousarray(np.transpose(w, (1, 2, 0)))  # (Cin,3,Cout)
        com["b" + nm] = np.ascontiguousarray(np.stack([b, 0.02 * b], 1))
    for i, (nm, ci, co, _) in enumerate(FCS, start=1):
        w = np.asarray(inputs[f"fw{i}"])
        b = np.asarray(inputs[f"fb{i}"])
        com["w" + nm] = np.ascontiguousarray(w.T)  # (Cin, Cout)
        com["b" + nm] = np.ascontiguousarray(np.stack([b, 0.02 * b], 1))
    for i, (nm, ci, co) in enumerate(ATTS, start=1):
        w = np.asarray(inputs[f"aw{i}"])
        b = np.asarray(inputs[f"ab{i}"])
        wstk = np.zeros((96, co), np.float32)  # k-groups at 32-aligned rows
        for k in range(3):
            wstk[32 * k : 32 * k + ci, :] = w[:, :, k].T
        com["w" + nm] = wstk
        com["b" + nm] = np.ascontiguousarray(np.stack([b, 0.02 * b], 1))
    com["lwT"] = np.ascontiguousarray(np.asarray(inputs["lw"]).T)
    com["lb2"] = np.ascontiguousarray(np.asarray(inputs["lb"])[:, None])
    com["map"] = np.ascontiguousarray(
        np.asarray(inputs["mapping"])[0].reshape(NID, NE * S)
    )
    return com


_CACHE = {}


def _build(B_core, NB, num_devices):
    key = (B_core, NB, num_devices)
    if key in _CACHE:
        return _CACHE[key]
    nc = bacc.Bacc("TRN2", debug=False, num_devices=num_devices)
    io = {}
    io["x"] = nc.dram_tensor("x", [B_core, T, NA], FP, kind="ExternalInput").ap()
    io["idx"] = nc.dram_tensor("idx", [B_core], mybir.dt.int32, kind="ExternalInput").ap()
    io["map"] = nc.dram_tensor("map", [NID, NE * S], FP, kind="ExternalInput").ap()
    for nm, ci, co in CONVS:
        io["w" + nm] = nc.dram_tensor("w" + nm, [ci, 3, co], FP, kind="ExternalInput").ap()
        io["b" + nm] = nc.dram_tensor("b" + nm, [co, 2], FP, kind="ExternalInput").ap()
    for nm, ci, co, _ in FCS:
        io["w" + nm] = nc.dram_tensor("w" + nm, [ci, co], FP, kind="ExternalInput").ap()
        io["b" + nm] = nc.dram_tensor("b" + nm, [co, 2], FP, kind="ExternalInput").ap()
    for nm, ci, co in ATTS:
        io["w" + nm] = nc.dram_tensor("w" + nm, [96, co], FP, kind="ExternalInput").ap()
        io["b" + nm] = nc.dram_tensor("b" + nm, [co, 2], FP, kind="ExternalInput").ap()
    io["lwT"] = nc.dram_tensor("lwT", [T, T], FP, kind="ExternalInput").ap()
    io["lb2"] = nc.dram_tensor("lb2", [T, 1], FP, kind="ExternalInput").ap()
    io["out"] = nc.dram_tensor("out", [B_core, NE], FP, kind="ExternalOutput").ap()

    with tile.TileContext(nc) as tc:
        emit(tc, io, B_core, NB)
    nc.compile()
    _CACHE[key] = (nc, io)
    return nc, io


def kernel(**inputs):
    from concourse.bass_utils import run_bass_kernel_spmd

    x = np.ascontiguousarray(np.asarray(inputs["x"], dtype=np.float32))
    ids = np.ascontiguousarray(np.asarray(inputs["identity_id"], dtype=np.int32))
    B = x.shape[0]
    B_core = B // NCORES
    NB = 256
    com = prep_host_inputs(inputs, B_core)
    nc, _io = _build(B_core, NB, NCORES)
    in_maps = []
    for i in range(NCORES):
        m = dict(com)
        m["x"] = x[i * B_core : (i + 1) * B_core]
        m["idx"] = ids[i * B_core : (i + 1) * B_core]
        in_maps.append(m)
    trace = bool(int(os.environ.get("KERNEL_TRACE", "0")))
    kw = {}
    if trace:
        kw = dict(trace=True, tmpdir=os.environ.get("KERNEL_TRACE_DIR") or None)
    res = run_bass_kernel_spmd(nc, in_maps, list(range(NCORES)), **kw)
    global _LAST_RESULTS
    _LAST_RESULTS = res
    out = np.concatenate([res.results[i]["out"] for i in range(NCORES)], axis=0)
    return out.astype(np.float32)


_LAST_RESULTS = None


# revision 13
# speedup vs baseline: 11.9902x; 11.9902x over previous
"""Trainium2 Bass kernel for nn_ExpressionEstimator_Attention.

Data-parallel across 8 NeuronCores: each core processes B/8 = 4096 samples.
Per-core pipeline (activations kept as (C_partitions, b, t) SBUF tiles):
  x load -> PE transpose to channel-major -> conv1..4 (per-t matmuls, 3 taps
  accumulated in PSUM) -> DRAM-roundtrip reshape (the torch-faithful
  (B,64,9)->(B*9,64) row-major remix) -> fc1..3 -> att conv stack (taps
  K-stacked via shifted SBUF replicas, one matmul per t) -> logits matmul ->
  exp -> transpose to batch-partition layout -> softmax-normalized attention
  contraction -> indirect-DMA gather of mapping rows -> final einsum on DVE.

Leaky ReLU (exact, no Lrelu table): r = Identity(0.02*psum + 0.02*b) on ACT,
out = (psum + b) max r on DVE — one op per engine, bias via per-partition APs.
"""

import os
import sys

for _p in ("/opt/trn_rl_repo", "/root/.axon_site/_ro/trn_rl_repo"):
    if os.path.isdir(_p) and _p not in sys.path:
        sys.path.insert(0, _p)

import numpy as np

import concourse.bass as bass
import concourse.mybir as mybir
import concourse.tile as tile
from concourse import bacc
from concourse.masks import make_identity

FP = mybir.dt.float32
AX = mybir.AxisListType.X
OP = mybir.AluOpType
AF = mybir.ActivationFunctionType

NCORES = 8
B_FULL = 32768
T = 9
NA = 64
S = 32
NE = 53
NID = 5000

# (name, Cin, Cout) for the main conv stack
CONVS = [("c1", 64, 32), ("c2", 32, 64), ("c3", 64, 128), ("c4", 128, 64)]
FCS = [("f1", 64, 128, "leaky"), ("f2", 128, 64, "leaky"), ("f3", 64, 32, "tanh")]
ATTS = [("a1", 32, 16), ("a2", 16, 8), ("a3", 8, 4), ("a4", 4, 2), ("a5", 2, 1)]
WINDOWS = [(0, 2), (2, 2), (4, 2), (6, 2), (8, 1)]


def leaky_act(nc, rt, pm, out, bias2):
    """out = leaky(pm + b)  with bias2 = [[b, 0.02b]] per-partition.

    rt (SBUF scratch) <- ACT Identity(0.02*pm + 0.02*b)
    out <- DVE (pm + b) max rt
    """
    nc.scalar.activation(rt, pm, AF.Identity, scale=0.02, bias=bias2[:, 1:2])
    nc.vector.scalar_tensor_tensor(
        out=out, in0=pm, scalar=bias2[:, 0:1], in1=rt, op0=OP.add, op1=OP.max
    )


def emit(tc, io, B_core, NB, repeat=1):
    """Emit the per-core program. io: dict name -> DRAM AP."""
    nc = tc.nc
    NCH = B_core // NB
    J = NB * T // 128
    assert NB * T % 128 == 0 and NB % 128 == 0
    NBU = NB // 128
    FLAT = NB * T

    import contextlib

    with contextlib.ExitStack() as ctx:
        const = ctx.enter_context(tc.tile_pool(name="const", bufs=1))
        xr_p = ctx.enter_context(tc.tile_pool(name="xr", bufs=2))
        st_p = ctx.enter_context(tc.tile_pool(name="stage", bufs=6))
        rt_p = ctx.enter_context(tc.tile_pool(name="rt", bufs=3))
        sm_p = ctx.enter_context(tc.tile_pool(name="small", bufs=2))
        m_p = ctx.enter_context(tc.tile_pool(name="gath", bufs=2))
        ob_p = ctx.enter_context(tc.tile_pool(name="outb", bufs=2))
        dram = ctx.enter_context(tc.tile_pool(name="dram", bufs=2, space="DRAM"))
        rep_ps = {
            nm: ctx.enter_context(tc.tile_pool(name="repp_" + nm, bufs=1))
            for nm, _, _ in ATTS
        }
        ps_tr = ctx.enter_context(tc.tile_pool(name="ps_tr", bufs=2, space="PSUM"))
        ps_mm = ctx.enter_context(tc.tile_pool(name="ps_mm", bufs=3, space="PSUM"))
        ps_t2 = ctx.enter_context(tc.tile_pool(name="ps_t2", bufs=2, space="PSUM"))

        # ---------------- constants ----------------
        ident = const.tile([128, 128], FP)
        make_identity(nc, ident[:])

        wt = {}
        bt = {}
        for nm, ci, co in CONVS:
            wt[nm] = const.tile([ci, 3, co], FP, name="w_" + nm)
            nc.sync.dma_start(wt[nm][:], io["w" + nm][:])
            bt[nm] = const.tile([co, 2], FP, name="b_" + nm)
            nc.sync.dma_start(bt[nm][:], io["b" + nm][:])
        for nm, ci, co, _ in FCS:
            wt[nm] = const.tile([ci, co], FP, name="w_" + nm)
            nc.sync.dma_start(wt[nm][:], io["w" + nm][:])
            bt[nm] = const.tile([co, 2], FP, name="b_" + nm)
            nc.sync.dma_start(bt[nm][:], io["b" + nm][:])
        for nm, ci, co in ATTS:
            wt[nm] = const.tile([96, co], FP, name="w_" + nm)
            nc.sync.dma_start(wt[nm][:], io["w" + nm][:])
            bt[nm] = const.tile([co, 2], FP, name="b_" + nm)
            nc.sync.dma_start(bt[nm][:], io["b" + nm][:])
        lw_t = const.tile([T, T], FP)
        nc.sync.dma_start(lw_t[:], io["lwT"][:])
        lb_t = const.tile([T, 1], FP)
        nc.sync.dma_start(lb_t[:], io["lb2"][:])

        idx_t = const.tile([128, NCH * NBU], mybir.dt.int32)
        nc.sync.dma_start(
            idx_t[:],
            io["idx"].rearrange("(col p) -> p col", p=128),
        )

        x_rows = io["x"].rearrange("b t c -> (b t) c")
        out_d = io["out"]
        map_d = io["map"]

        # ---------------- chunk loop ----------------
        for c in [cc for _r in range(repeat) for cc in range(NCH)]:
            # -- load x chunk, rows (b t) on partitions --
            xr = xr_p.tile([128, J, NA], FP)
            nc.sync.dma_start(
                xr[:],
                x_rows[c * FLAT : (c + 1) * FLAT, :].rearrange(
                    "(p j) c -> p j c", p=128
                ),
            )
            # -- transpose to X0 (64, t, b): pair (j, j+9) -> (128,128) --
            x0 = st_p.tile([NA, T, NB], FP, tag="stage")
            for t in range(T):
                ptr = ps_tr.tile([128, 128], FP)
                pair = xr[:, t :: T, :].rearrange("p q c -> p (q c)")
                nc.tensor.transpose(ptr[:], pair, ident[:128, :128])
                nc.scalar.copy(out=x0[:, t, 0::NBU], in_=ptr[:NA, :])
                nc.vector.tensor_copy(x0[:, t, 1::NBU], ptr[NA:, :])

            # -- main convs: 2-t windowed matmuls (N=2*NB<=512) --
            cur = x0
            for nm, ci, co in CONVS:
                nxt = st_p.tile([co, T, NB], FP, tag="stage")
                for t0, L in WINDOWS:
                    pm = ps_mm.tile([co, 2, NB], FP, tag="pm")
                    full = [k for k in range(3) if 0 <= t0 + k - 1 and t0 + k - 1 + L <= T]
                    part = [k for k in range(3) if k not in full]
                    for i, k in enumerate(full):
                        nc.tensor.matmul(
                            pm[:, :L, :], wt[nm][:, k, :],
                            cur[:, t0 + k - 1 : t0 + k - 1 + L, :],
                            start=(i == 0), stop=(i == len(full) - 1 and not part),
                        )
                    for j, k in enumerate(part):
                        # only interior window positions are valid for this tap
                        lo = max(0, 1 - k - t0)
                        hi = min(L, T + 1 - k - t0)
                        if lo >= hi:
                            continue
                        nc.tensor.matmul(
                            pm[:, lo:hi, :], wt[nm][:, k, :],
                            cur[:, t0 + lo + k - 1 : t0 + hi + k - 1, :],
                            start=False, stop=(j == len(part) - 1),
                        )
                    rt = rt_p.tile([128, 2 * NB], FP, tag="rt")
                    leaky_act(
                        nc, rt[:co, : L * NB],
                        pm[:, :L, :].rearrange("c l b -> c (l b)"),
                        nxt[:, t0 : t0 + L, :].rearrange("c l b -> c (l b)"),
                        bt[nm],
                    )
                cur = nxt

            # -- reshape via DRAM roundtrip: D2 row i = 9c+t = 64r+j --
            d2 = dram.tile([NA * T, NB], FP)
            nc.sync.dma_start(
                d2[:].rearrange("(c t) b -> c (t b)", t=T),
                cur[:].rearrange("c t b -> c (t b)"),
            )
            z = st_p.tile([64, T, NB], FP, tag="stage")
            nc.sync.dma_start(
                z[:],
                d2[:].rearrange("(r j) b -> j r b", j=64),
            )

            # -- fc stack on flat (r b) columns --
            curf = z[:].rearrange("j r b -> j (r b)")
            for nm, ci, co, act in FCS:
                nxt = st_p.tile([co, FLAT], FP, tag="stage")
                for q0 in range(0, FLAT, 512):
                    q1 = min(q0 + 512, FLAT)
                    pmt = ps_mm.tile([co, 512], FP, tag="pm")
                    pm = pmt[:, : q1 - q0]
                    nc.tensor.matmul(pm, wt[nm][:], curf[:, q0:q1], start=True, stop=True)
                    if act == "leaky":
                        rt = rt_p.tile([128, 512], FP, tag="rt")
                        leaky_act(nc, rt[:co, : q1 - q0], pm, nxt[:, q0:q1], bt[nm])
                    else:
                        nc.scalar.activation(
                            nxt[:, q0:q1], pm, AF.Tanh, bias=bt[nm][:, 0:1]
                        )
                curf = nxt[:]

            rs = curf.rearrange("s (r b) -> s r b", b=NB)  # (32, 9, NB)

            # -- attention convs: shifted K-stacked replicas, 1 matmul per t --
            prev = rs
            for nm, ci, co in ATTS:
                rep = rep_ps[nm].tile([96, T, NB], FP, tag="rep")
                if c == 0:
                    # zero whole tile once (boundary slices + inter-k pad rows);
                    # this pool slot is exclusive to this layer so zeros persist.
                    nc.gpsimd.memset(rep[:], 0.0)
                nc.sync.dma_start(rep[0:ci, 1:T, :], prev[:, 0 : T - 1, :])
                nc.sync.dma_start(rep[32 : 32 + ci, :, :], prev[:, :, :])
                nc.sync.dma_start(rep[64 : 64 + ci, 0 : T - 1, :], prev[:, 1:T, :])
                nxt = st_p.tile([co, T, NB], FP, tag="stage")
                for t0, L in WINDOWS:
                    pm = ps_mm.tile([co, 2, NB], FP, tag="pm")
                    nc.tensor.matmul(
                        pm[:, :L, :], wt[nm][:], rep[:, t0 : t0 + L, :],
                        start=True, stop=True,
                    )
                    rt = rt_p.tile([128, 2 * NB], FP, tag="rt")
                    leaky_act(
                        nc, rt[:co, : L * NB],
                        pm[:, :L, :].rearrange("c l b -> c (l b)"),
                        nxt[:, t0 : t0 + L, :].rearrange("c l b -> c (l b)"),
                        bt[nm],
                    )
                prev = nxt[:]

            # -- logits: gather a into (9, NB) then one matmul --
            a5t = sm_p.tile([T, NB], FP, tag="a5t")
            nc.sync.dma_start(a5t[:], prev)
            pml = ps_mm.tile([T, NB], FP, tag="pm")
            nc.tensor.matmul(pml[:], lw_t[:], a5t[:], start=True, stop=True)
            et9 = sm_p.tile([T, NB], FP, tag="exp9")
            nc.scalar.activation(et9[:], pml[:], AF.Exp, bias=lb_t[:, 0:1])

            # -- per-128 subchunk: transpose to b-partitions, attention + output --
            for u in range(NBU):
                bsl = slice(u * 128, (u + 1) * 128)
                prt = ps_t2.tile([128, T, S], FP, tag="t2")
                for t in range(T):
                    nc.tensor.transpose(prt[:, t, :], rs[:, t, bsl], ident[:S, :S])
                pre = ps_t2.tile([128, T], FP, tag="t2")
                nc.tensor.transpose(pre[:], et9[:, bsl], ident[:T, :T])

                rsb = sm_p.tile([128, T, S], FP, tag="rsb")
                nc.vector.tensor_copy(rsb[:], prt[:])
                etb = sm_p.tile([128, T], FP, tag="etb")
                nc.scalar.copy(out=etb[:], in_=pre[:])

                den = sm_p.tile([128, 1], FP, tag="den")
                nc.gpsimd.tensor_reduce(out=den[:], in_=etb[:], op=OP.add, axis=AX)
                rcp = sm_p.tile([128, 1], FP, tag="rcp")
                nc.vector.reciprocal(rcp[:], den[:])

                pp = sm_p.tile([128, S, T], FP, tag="pp")
                nc.vector.tensor_tensor(
                    out=pp[:].rearrange("p s t -> p t s"),
                    in0=rsb[:],
                    in1=etb[:].unsqueeze(2).broadcast_to([128, T, S]),
                    op=OP.mult,
                )
                sub_u = sm_p.tile([128, S], FP, tag="subu")
                nc.gpsimd.tensor_reduce(out=sub_u[:], in_=pp[:], op=OP.add, axis=AX)
                sub_n = sm_p.tile([128, S], FP, tag="subn")
                nc.vector.tensor_scalar(
                    out=sub_n[:], in0=sub_u[:], scalar1=rcp[:, 0:1], scalar2=10.0,
                    op0=OP.mult, op1=OP.mult,
                )

                mt = m_p.tile([128, NE * S], FP, tag="mt")
                nc.gpsimd.indirect_dma_start(
                    out=mt[:],
                    out_offset=None,
                    in_=map_d[:],
                    in_offset=bass.IndirectOffsetOnAxis(
                        ap=idx_t[:, c * NBU + u : c * NBU + u + 1], axis=0
                    ),
                )
                pf = sm_p.tile([128, NE, S], FP, tag="pf")
                nc.vector.tensor_tensor(
                    out=pf[:],
                    in0=mt[:].rearrange("p (e s) -> p e s", s=S),
                    in1=sub_n[:].unsqueeze(1).broadcast_to([128, NE, S]),
                    op=OP.mult,
                )
                ob = ob_p.tile([128, NE], FP, tag="ob")
                nc.gpsimd.tensor_reduce(out=ob[:], in_=pf[:], op=OP.add, axis=AX)
                nc.sync.dma_start(out_d[c * NB + u * 128 : c * NB + (u + 1) * 128, :], ob[:])


def prep_host_inputs(inputs, B_core):
    """Transform weights to device layouts. Returns (common dict, per-core fn)."""
    com = {}
    for i, (nm, ci, co) in enumerate(CONVS, start=1):
        w = np.asarray(inputs[f"cw{i}"])
        b = np.asarray(inputs[f"cb{i}"])
        com["w" + nm] = np.ascontiguousarray(np.transpose(w, (1, 2, 0)))  # (Cin,3,Cout)
        com["b" + nm] = np.ascontiguousarray(np.stack([b, 0.02 * b], 1))
    for i, (nm, ci, co, _) in enumerate(FCS, start=1):
        w = np.asarray(inputs[f"fw{i}"])
        b = np.asarray(inputs[f"fb{i}"])
        com["w" + nm] = np.ascontiguousarray(w.T)  # (Cin, Cout)
        com["b" + nm] = np.ascontiguousarray(np.stack([b, 0.02 * b], 1))
    for i, (nm, ci, co) in enumerate(ATTS, start=1):
        w = np.asarray(inputs[f"aw{i}"])
        b = np.asarray(inputs[f"ab{i}"])
        wstk = np.zeros((96, co), np.float32)  # k-groups at 32-aligned rows
        for k in range(3):
            wstk[32 * k : 32 * k + ci, :] = w[:, :, k].T
        com["w" + nm] = wstk
        com["b" + nm] = np.ascontiguousarray(np.stack([b, 0.02 * b], 1))
    com["lwT"] = np.ascontiguousarray(np.asarray(inputs["lw"]).T)
    com["lb2"] = np.ascontiguousarray(np.asarray(inputs["lb"])[:, None])
    com["map"] = np.ascontiguousarray(
        np.asarray(inputs["mapping"])[0].reshape(NID, NE * S)
    )
    return com


_CACHE = {}


def _build(B_core, NB, num_devices, repeat=1):
    key = (B_core, NB, num_devices, repeat)
    if key in _CACHE:
        return _CACHE[key]
    nc = bacc.Bacc("TRN2", debug=False, num_devices=num_devices)
    io = {}
    io["x"] = nc.dram_tensor("x", [B_core, T, NA], FP, kind="ExternalInput").ap()
    io["idx"] = nc.dram_tensor("idx", [B_core], mybir.dt.int32, kind="ExternalInput").ap()
    io["map"] = nc.dram_tensor("map", [NID, NE * S], FP, kind="ExternalInput").ap()
    for nm, ci, co in CONVS:
        io["w" + nm] = nc.dram_tensor("w" + nm, [ci, 3, co], FP, kind="ExternalInput").ap()
        io["b" + nm] = nc.dram_tensor("b" + nm, [co, 2], FP, kind="ExternalInput").ap()
    for nm, ci, co, _ in FCS:
        io["w" + nm] = nc.dram_tensor("w" + nm, [ci, co], FP, kind="ExternalInput").ap()
        io["b" + nm] = nc.dram_tensor("b" + nm, [co, 2], FP, kind="ExternalInput").ap()
    for nm, ci, co in ATTS:
        io["w" + nm] = nc.dram_tensor("w" + nm, [96, co], FP, kind="ExternalInput").ap()
        io["b" + nm] = nc.dram_tensor("b" + nm, [co, 2], FP, kind="ExternalInput").ap()
    io["lwT"] = nc.dram_tensor("lwT", [T, T], FP, kind="ExternalInput").ap()
    io["lb2"] = nc.dram_tensor("lb2", [T, 1], FP, kind="ExternalInput").ap()
    io["out"] = nc.dram_tensor("out", [B_core, NE], FP, kind="ExternalOutput").ap()

    with tile.TileContext(nc) as tc:
        emit(tc, io, B_core, NB, repeat=repeat)
    nc.compile()
    _CACHE[key] = (nc, io)
    return nc, io


def kernel(**inputs):
    from concourse.bass_utils import run_bass_kernel_spmd

    x = np.ascontiguousarray(np.asarray(inputs["x"], dtype=np.float32))
    ids = np.ascontiguousarray(np.asarray(inputs["identity_id"], dtype=np.int32))
    B = x.shape[0]
    B_core = B // NCORES
    NB = 256
    com = prep_host_inputs(inputs, B_core)
    nc, _io = _build(B_core, NB, NCORES)
    in_maps = []
    for i in range(NCORES):
        m = dict(com)
        m["x"] = x[i * B_core : (i + 1) * B_core]
        m["idx"] = ids[i * B_core : (i + 1) * B_core]
        in_maps.append(m)
    trace = bool(int(os.environ.get("KERNEL_TRACE", "0")))
    kw = {}
    if trace:
        kw = dict(trace=True, tmpdir=os.environ.get("KERNEL_TRACE_DIR") or None)
    res = run_bass_kernel_spmd(nc, in_maps, list(range(NCORES)), **kw)
    global _LAST_RESULTS
    _LAST_RESULTS = res
    out = np.concatenate([res.results[i]["out"] for i in range(NCORES)], axis=0)
    return out.astype(np.float32)


_LAST_RESULTS = None
